# revision 6
# baseline (speedup 1.0000x reference)
"""DropStripes (SpecAugment freq-stripe dropout) Trainium2 kernel.

y[b, t, f] = x[b, t, f] * keep[b, f] where keep masks out up to 2 random
stripes (width < 64) along the last (freq, W=512) axis, derived from
uniform samples u_dist/u_bgn exactly as the reference does:
    dist = floor(u_dist * 64);  bgn = floor(u_bgn * (512 - dist))
    in_stripe = bgn <= f < bgn + dist;  keep = !any_stripe

Sharding: pure data-parallel over batch (64 = 8 cores x 8 samples), no
communication. Per core: 32 MB in + 32 MB out of HBM traffic (memory-bound);
the mask math is a handful of tiny DVE ops.

Per-core layout: partitions = time (128 rows), free = freq (contiguous 2 KB
rows in DRAM -> efficient DMA). The [8, 512] keep mask is computed on 8
partitions, reshaped to one row via SBUF->SBUF DMA, broadcast to all 128
partitions, and each 4 MB sample tile is multiplied in-place by the mask
(free-dim step-0 broadcast AP) before being stored back.

floor() is built from IEEE ops only (round-to-int via +-2^23 magic, then
subtract (r > x)), so the mask matches the f32 reference bit-exactly.
"""

import numpy as np

import concourse.bacc as bacc
import concourse.mybir as mybir
from concourse.bass import broadcast_tensor_aps
from concourse.mybir import AluOpType as Op
from concourse.tile import TileContext

N_CORES = 8
B = 64          # full batch
BP = B // N_CORES  # batch per core = 8
T = 2048
F = 512
DROP_WIDTH = 64
S = 2           # stripes per sample
P = 128         # SBUF partitions
O = T // P      # 16 time sub-tiles per sample
MAGIC = 8388608.0  # 2^23: x + MAGIC - MAGIC == round-to-nearest-even(x) for 0<=x<2^22

F32 = mybir.dt.float32


def build_nc():
    nc = bacc.Bacc("TRN2", target_bir_lowering=False)
    x = nc.dram_tensor("x", [BP, T, F], F32, kind="ExternalInput")
    u_dist = nc.dram_tensor("u_dist", [BP, S], F32, kind="ExternalInput")
    u_bgn = nc.dram_tensor("u_bgn", [BP, S], F32, kind="ExternalInput")
    y = nc.dram_tensor("y", [BP, T, F], F32, kind="ExternalOutput")

    v = nc.vector

    with TileContext(nc) as tc:
        with (
            tc.tile_pool(name="mask", bufs=1) as mp,
            tc.tile_pool(name="data", bufs=3) as dp,
        ):
            def mtile(shape, tag):
                return mp.tile(shape, F32, tag=tag, name=tag)

            # ---- tiny per-(sample, stripe) scalars, on BP partitions ----
            ud = mtile([BP, S], "ud")
            ub = mtile([BP, S], "ub")
            nc.sync.dma_start(out=ud[:, :], in_=u_dist[:, :])
            nc.sync.dma_start(out=ub[:, :], in_=u_bgn[:, :])

            def floor_(out_t, in_ap, scratch_tag):
                # r = RNE(x); floor = r - (r > x)
                r = mtile(list(in_ap.shape), scratch_tag + "_r")
                g = mtile(list(in_ap.shape), scratch_tag + "_g")
                v.tensor_scalar(r[:, :], in_ap, MAGIC, MAGIC, Op.add, Op.subtract)
                v.tensor_tensor(g[:, :], r[:, :], in_ap, Op.is_gt)
                v.tensor_tensor(out_t[:, :], r[:, :], g[:, :], Op.subtract)

            w = mtile([BP, S], "w")
            v.tensor_scalar(w[:, :], ud[:, :], float(DROP_WIDTH), None, Op.mult)
            dist = mtile([BP, S], "dist")
            floor_(dist, w[:, :], "fd")

            avail = mtile([BP, S], "avail")  # 512 - dist
            v.tensor_scalar(avail[:, :], dist[:, :], -1.0, float(F), Op.mult, Op.add)
            vb = mtile([BP, S], "vb")        # u_bgn * (512 - dist)
            v.tensor_tensor(vb[:, :], ub[:, :], avail[:, :], Op.mult)
            bgn = mtile([BP, S], "bgn")
            floor_(bgn, vb[:, :], "fb")
            end = mtile([BP, S], "end")
            v.tensor_tensor(end[:, :], bgn[:, :], dist[:, :], Op.add)

            # ---- keep mask [BP, F] ----
            io = mtile([BP, F], "io")
            nc.gpsimd.iota(
                io[:, :], pattern=[[1, F]], base=0, channel_multiplier=0,
                allow_small_or_imprecise_dtypes=True,
            )
            hit = []
            for s in range(S):
                lt = mtile([BP, F], f"lt{s}")
                v.tensor_scalar(lt[:, :], io[:, :], end[:, s : s + 1], None, Op.is_lt)
                h = mtile([BP, F], f"hit{s}")
                # (io >= bgn_s) * lt
                v.scalar_tensor_tensor(
                    h[:, :], io[:, :], bgn[:, s : s + 1], lt[:, :], Op.is_ge, Op.mult
                )
                hit.append(h)
            nk = mtile([BP, F], "nk")
            v.tensor_tensor(nk[:, :], hit[0][:, :], hit[1][:, :], Op.max)
            keep = mtile([BP, F], "keep")
            v.tensor_scalar(keep[:, :], nk[:, :], -1.0, 1.0, Op.mult, Op.add)

            # ---- replicate to all 128 partitions: mask[p, b*F + f] ----
            krow = mtile([1, BP * F], "krow")
            nc.sync.dma_start(out=krow[:, :], in_=keep[:, :])
            mask = mtile([P, BP * F], "mask")
            nc.gpsimd.partition_broadcast(mask[:, :], krow[:1, :], channels=P)

            # ---- stream x through SBUF, multiply, store ----
            # t = p*O + j: each partition's row is one contiguous 32 KB DRAM
            # block -> 128 big descriptors per DMA instead of 2048 small ones.
            for b in range(BP):
                tl = dp.tile([P, O * F], F32, tag="tl", name=f"tl{b}")
                tl3 = tl[:, :].rearrange("p (j f) -> p j f", f=F)
                src = x[b].rearrange("(p j) f -> p j f", p=P)
                nc.sync.dma_start(out=tl3, in_=src)

                m3 = mask[:, b * F : (b + 1) * F].rearrange("p (o f) -> p o f", o=1)
                a3, m3b = broadcast_tensor_aps(tl3, m3)
                v.tensor_tensor(tl3, a3, m3b, Op.mult)

                dst = y[b].rearrange("(p j) f -> p j f", p=P)
                nc.scalar.dma_start(out=dst, in_=tl3)

    nc.compile()
    return nc


_NC = None


def _get_nc():
    global _NC
    if _NC is None:
        _NC = build_nc()
    return _NC


def _shard(a):
    return [np.ascontiguousarray(a[c * BP : (c + 1) * BP]) for c in range(N_CORES)]


def run(x, u_dist, u_bgn, trace=False):
    from concourse.bass_utils import run_bass_kernel_spmd

    nc = _get_nc()
    xs, uds, ubs = _shard(x), _shard(u_dist), _shard(u_bgn)
    in_maps = [
        {"x": xs[c], "u_dist": uds[c], "u_bgn": ubs[c]} for c in range(N_CORES)
    ]
    res = run_bass_kernel_spmd(
        nc, in_maps, core_ids=list(range(N_CORES)), trace=trace
    )
    out = np.concatenate([res.results[c]["y"] for c in range(N_CORES)], axis=0)
    return out, res


def kernel(x, u_dist, u_bgn):
    x = np.asarray(x, dtype=np.float32)
    u_dist = np.asarray(u_dist, dtype=np.float32)
    u_bgn = np.asarray(u_bgn, dtype=np.float32)
    out, _ = run(x, u_dist, u_bgn, trace=False)
    return out


# revision 15
# speedup vs baseline: 1.4081x; 1.4081x over previous
"""DropStripes (SpecAugment freq-stripe dropout) Trainium2 kernel.

y[b, t, f] = x[b, t, f] * keep[b, f] where keep masks out up to 2 random
stripes (width < 64) along the last (freq, W=512) axis, derived from
uniform samples u_dist/u_bgn exactly as the reference does:
    dist = floor(u_dist * 64);  bgn = floor(u_bgn * (512 - dist))
    in_stripe = bgn <= f < bgn + dist;  keep = !any_stripe

Sharding: pure data-parallel over batch (64 = 8 cores x 8 samples), no
communication. Per core: 32 MB in + 32 MB out of HBM traffic (memory-bound);
the mask math is a handful of tiny DVE ops.

Per-core layout: partitions = time (128 rows), free = freq (contiguous 2 KB
rows in DRAM -> efficient DMA). The [8, 512] keep mask is computed on 8
partitions, reshaped to one row via SBUF->SBUF DMA, broadcast to all 128
partitions, and each 4 MB sample tile is multiplied in-place by the mask
(free-dim step-0 broadcast AP) before being stored back.

floor() is built from IEEE ops only (round-to-int via +-2^23 magic, then
subtract (r > x)), so the mask matches the f32 reference bit-exactly.
"""

import numpy as np

import concourse.bacc as bacc
import concourse.mybir as mybir
from concourse.bass import broadcast_tensor_aps
from concourse.mybir import AluOpType as Op
from concourse.tile import TileContext

N_CORES = 8
B = 64          # full batch
BP = B // N_CORES  # batch per core = 8
T = 2048
F = 512
DROP_WIDTH = 64
S = 2           # stripes per sample
P = 128         # SBUF partitions
O = T // P      # 16 time sub-tiles per sample
MAGIC = 8388608.0  # 2^23: x + MAGIC - MAGIC == round-to-nearest-even(x) for 0<=x<2^22

F32 = mybir.dt.float32


def build_nc(interleave=False, n_ch=1, bufs=4, store_eng="scalar", reps=1,
             limit_tiles=None):
    nc = bacc.Bacc("TRN2", target_bir_lowering=False)
    x = nc.dram_tensor("x", [BP, T, F], F32, kind="ExternalInput")
    u_dist = nc.dram_tensor("u_dist", [BP, S], F32, kind="ExternalInput")
    u_bgn = nc.dram_tensor("u_bgn", [BP, S], F32, kind="ExternalInput")
    y = nc.dram_tensor("y", [BP, T, F], F32, kind="ExternalOutput")

    v = nc.vector

    with TileContext(nc) as tc:
        with (
            tc.tile_pool(name="mask", bufs=1) as mp,
            tc.tile_pool(name="data", bufs=bufs) as dp,
        ):
            def mtile(shape, tag):
                return mp.tile(shape, F32, tag=tag, name=tag)

            # ---- tiny per-(sample, stripe) scalars, on BP partitions ----
            ud = mtile([BP, S], "ud")
            ub = mtile([BP, S], "ub")
            nc.sync.dma_start(out=ud[:, :], in_=u_dist[:, :])
            nc.sync.dma_start(out=ub[:, :], in_=u_bgn[:, :])

            def floor_(out_t, in_ap, scratch_tag):
                # r = RNE(x); floor = r - (r > x)
                r = mtile(list(in_ap.shape), scratch_tag + "_r")
                g = mtile(list(in_ap.shape), scratch_tag + "_g")
                v.tensor_scalar(r[:, :], in_ap, MAGIC, MAGIC, Op.add, Op.subtract)
                v.tensor_tensor(g[:, :], r[:, :], in_ap, Op.is_gt)
                v.tensor_tensor(out_t[:, :], r[:, :], g[:, :], Op.subtract)

            w = mtile([BP, S], "w")
            v.tensor_scalar(w[:, :], ud[:, :], float(DROP_WIDTH), None, Op.mult)
            dist = mtile([BP, S], "dist")
            floor_(dist, w[:, :], "fd")

            avail = mtile([BP, S], "avail")  # 512 - dist
            v.tensor_scalar(avail[:, :], dist[:, :], -1.0, float(F), Op.mult, Op.add)
            vb = mtile([BP, S], "vb")        # u_bgn * (512 - dist)
            v.tensor_tensor(vb[:, :], ub[:, :], avail[:, :], Op.mult)
            bgn = mtile([BP, S], "bgn")
            floor_(bgn, vb[:, :], "fb")
            end = mtile([BP, S], "end")
            v.tensor_tensor(end[:, :], bgn[:, :], dist[:, :], Op.add)

            # ---- keep mask [BP, F] ----
            io = mtile([BP, F], "io")
            nc.gpsimd.iota(
                io[:, :], pattern=[[1, F]], base=0, channel_multiplier=0,
                allow_small_or_imprecise_dtypes=True,
            )
            hit = []
            for s in range(S):
                lt = mtile([BP, F], f"lt{s}")
                v.tensor_scalar(lt[:, :], io[:, :], end[:, s : s + 1], None, Op.is_lt)
                h = mtile([BP, F], f"hit{s}")
                # (io >= bgn_s) * lt
                v.scalar_tensor_tensor(
                    h[:, :], io[:, :], bgn[:, s : s + 1], lt[:, :], Op.is_ge, Op.mult
                )
                hit.append(h)
            nk = mtile([BP, F], "nk")
            v.tensor_tensor(nk[:, :], hit[0][:, :], hit[1][:, :], Op.max)
            keep = mtile([BP, F], "keep")
            v.tensor_scalar(keep[:, :], nk[:, :], -1.0, 1.0, Op.mult, Op.add)

            # ---- replicate to all 128 partitions: mask[p, b*F + f] ----
            krow = mtile([1, BP * F], "krow")
            nc.sync.dma_start(out=krow[:, :], in_=keep[:, :])
            mask = mtile([P, BP * F], "mask")
            nc.gpsimd.partition_broadcast(mask[:, :], krow[:1, :], channels=P)

            # ---- stream x through SBUF, multiply, store ----
            # Contiguous layout (default): t = p*O + j, so each partition's
            # row is one contiguous DRAM block -> 128 big descriptors per
            # DMA. Interleaved: t = j*128 + p (2 KB strided descriptors).
            store = getattr(nc, store_eng)
            R = O // n_ch  # time sub-rows per DMA chunk
            n_done = 0
            for rep in range(reps):  # reps>1: benchmarking only (idempotent)
                for b in range(BP):
                    if limit_tiles is not None and n_done >= limit_tiles:
                        break
                    n_done += 1
                    if interleave:
                        xb = x[b].rearrange("(j p) f -> p j f", p=P)
                        yb = y[b].rearrange("(j p) f -> p j f", p=P)
                    else:
                        xb = x[b].rearrange("(p j) f -> p j f", p=P)
                        yb = y[b].rearrange("(p j) f -> p j f", p=P)
                    for c in range(n_ch):
                        tl = dp.tile(
                            [P, R * F], F32, tag="tl", name=f"tl{rep}_{b}_{c}"
                        )
                        tl3 = tl[:, :].rearrange("p (j f) -> p j f", f=F)
                        src = xb[:, c * R : (c + 1) * R, :]
                        nc.sync.dma_start(out=tl3, in_=src)

                        m3 = mask[:, b * F : (b + 1) * F].rearrange(
                            "p (o f) -> p o f", o=1
                        )
                        a3, m3b = broadcast_tensor_aps(tl3, m3)
                        v.tensor_tensor(tl3, a3, m3b, Op.mult)

                        store.dma_start(out=yb[:, c * R : (c + 1) * R, :], in_=tl3)

    nc.compile()
    return nc


_NC = None


def _get_nc():
    global _NC
    if _NC is None:
        _NC = build_nc()
    return _NC


def _shard(a):
    return [np.ascontiguousarray(a[c * BP : (c + 1) * BP]) for c in range(N_CORES)]


def _stub_axon_hooks():
    # bass_utils' trace path imports antenv.axon_hooks, which some axon
    # installs lack; a None hook makes it fall back to untraced execution
    # instead of crashing (e.g. if BASS_TRACE=1 is set in the environment).
    import sys
    import types

    try:
        import antenv.axon_hooks  # noqa: F401
    except ImportError:
        m = types.ModuleType("antenv.axon_hooks")
        m.get_axon_ntff_profile_hook = lambda: None
        sys.modules["antenv.axon_hooks"] = m


def run(x, u_dist, u_bgn, trace=False):
    _stub_axon_hooks()
    from concourse.bass_utils import run_bass_kernel_spmd

    nc = _get_nc()
    xs, uds, ubs = _shard(x), _shard(u_dist), _shard(u_bgn)
    in_maps = [
        {"x": xs[c], "u_dist": uds[c], "u_bgn": ubs[c]} for c in range(N_CORES)
    ]
    res = run_bass_kernel_spmd(
        nc, in_maps, core_ids=list(range(N_CORES)), trace=trace
    )
    out = np.concatenate([res.results[c]["y"] for c in range(N_CORES)], axis=0)
    return out, res


def kernel(x, u_dist, u_bgn):
    x = np.asarray(x, dtype=np.float32)
    u_dist = np.asarray(u_dist, dtype=np.float32)
    u_bgn = np.asarray(u_bgn, dtype=np.float32)
    out, _ = run(x, u_dist, u_bgn, trace=False)
    return out
